# revision 38
# baseline (speedup 1.0000x reference)
"""Trainium2 Bass kernel: scatter flat upper-triangular values into dense
[B, 2048, 2048] matrices (zeros below the diagonal).

Strategy (pure data parallel, 4 samples per core on 8 cores; default
mode "fusedm"):

The padded output (OUT_NP = 2049*2048 floats per sample) is tiled
exactly by 2048 full-pitch "band rows" of width 2049: band row r =
[2049r, 2049(r+1)) = matrix row r's triu data (length 2048-r) followed
by matrix row r+1's zero prefix (length r+1). Band-row starts are
AFFINE (stride 2049) while the input triu row offsets are quadratic
(offset[r] = 2048r - r(r-1)/2), so per 128-row block k:

  1. four indirect-DMA gathers (one per sample, 128 descriptors each,
     per-row element offsets from an SBUF index table) fetch rows
     r in [128k, 128k+128) at fixed length L = 2048-128k into a
     [128, 4, 2049] tile (rows beyond their true length L-p pick up a
     junk tail);
  2. mask multiplies on DVE against a sliding master-mask slice zero
     the junk tail AND the zero suffix in-tile (only the tail
     [L-128, 2049) needs the pass; the prefix is always-valid data);
  3. one full-pitch band store per block (512 descriptors x 8196 B)
     writes the finished band rows at stride 2049.

Every output byte is written exactly once (no separate zero-fill DMAs,
no WAW ordering). Per core: 16 blocks x (4 gathers + 4 masks + 1
store) + consts ~= 150 instructions, ~103 MB HBM traffic, DMA-bound
(roofline ~287 us at 358 GB/s per-core HBM).
"""

import os
import sys

import numpy as np

for _p in ("/opt/trn_rl_repo", "/opt/pypackages"):
    if _p not in sys.path and os.path.isdir(_p):
        sys.path.append(_p)

MAT = 2048
P = 128                      # partitions / rows per block
NB = MAT // P                # 16 blocks
S = 4                        # samples per core
NCORES = 8
BATCH = S * NCORES           # 32
IN_N = MAT * (MAT + 1) // 2  # 2098176 triu elements per sample
PAD = 2048
FPAD = 128                   # front pad (grouped loads read up to H before row 0)
IN_NP = FPAD + IN_N + (PAD - FPAD)  # padded per-sample input length
OUT_N = MAT * MAT
OUT_NP = OUT_N + PAD         # padded per-sample output length
ZMAX = P * (NB - 1) + 1      # max zero-parallelogram row length (1921)
G = 16                       # rows per affine load group (grouped mode)
NG = P // G                  # 8 groups per block
H = (G - 1) * (G - 2) // 2   # 105: max residual head misalignment
WM = MAT + P * (NB - 1) + H + 7   # master mask width (4080)
WT = MAT + 1 + H             # band tile width in grouped mode (2154)
WM2 = P * (NB - 1) + MAT + 1  # fused master mask width (3969)

_row_off = None


def _offsets():
    global _row_off
    if _row_off is None:
        r = np.arange(MAT, dtype=np.int64)
        _row_off = r * MAT - r * (r - 1) // 2
    return _row_off


def _build_nc(repeat: int = 1, stages: str = "gmsz", fold: bool = False,
              bufs: int = 3, mode: str = "gather", leng: str = "pool",
              rep_loop: bool = False, store_split: int = 1):
    """stages: g=gathers/loads, m=mask, s=band stores, z=zero fills.
    mode: "gather" (indirect-DMA gather), "grouped" (affine group loads),
    or "fused" (folded gather + full-pitch select/store, no zero DMAs)."""
    import concourse.bass as bass
    import concourse.tile as tile
    from concourse import bacc, mybir

    off = _offsets()
    nc = bacc.Bacc("TRN2", target_bir_lowering=False, debug=False)
    inp = nc.dram_tensor("inp", [S * IN_NP, 1], mybir.dt.float32, kind="ExternalInput")
    idxt = nc.dram_tensor("idx", [P, NB * S], mybir.dt.int32, kind="ExternalInput")
    mskt = nc.dram_tensor("msk", [P, WM], mybir.dt.float32, kind="ExternalInput")
    out = nc.dram_tensor("out", [S * OUT_NP], mybir.dt.float32, kind="ExternalOutput")

    if mode == "grouped":
        return _build_grouped(nc, bass, tile, mybir, inp, mskt, out, off,
                              repeat, stages, bufs, leng)
    if mode in ("fused", "fusedm", "fusedi"):
        msk2 = None
        if mode in ("fusedm", "fusedi"):
            msk2 = nc.dram_tensor("msk2", [P, WM2], mybir.dt.float32,
                                  kind="ExternalInput")
        return _build_fused(nc, bass, tile, mybir, inp, idxt, out,
                            repeat, stages, bufs, fold, rep_loop, msk2,
                            store_split, ilv=(mode == "fusedi"))

    with tile.TileContext(nc) as tc:
        with (
            tc.tile_pool(name="band", bufs=bufs) as pool,
            tc.tile_pool(name="const", bufs=1) as cpool,
        ):
            idx_tile = cpool.tile([P, NB * S], mybir.dt.int32)
            nc.sync.dma_start(idx_tile[:], idxt[:, :])
            if "z" in stages:
                zt = cpool.tile([P, S * ZMAX], mybir.dt.float32)
                nc.vector.memset(zt[:], 0.0)
            def gblock(k):
                L = MAT - P * k
                t = pool.tile([P, S, L], mybir.dt.float32, tag="band")
                if "g" not in stages and "c" not in stages:
                    # ablation-only: init tile so the allocator/sim see a write
                    nc.vector.memset(t[:], 0.0)
                Lg = L // 4 if "q" in stages else L
                if "g" in stages:
                    if fold:
                        nc.gpsimd.indirect_dma_start(
                            out=t[:],
                            out_offset=None,
                            in_=inp[:],
                            in_offset=bass.IndirectOffsetOnAxis(
                                ap=idx_tile[:, k * S:(k + 1) * S], axis=0
                            ),
                        )
                    else:
                        for s in range(S):
                            nc.gpsimd.indirect_dma_start(
                                out=t[:, s, :Lg],
                                out_offset=None,
                                in_=inp[:],
                                in_offset=bass.IndirectOffsetOnAxis(
                                    ap=idx_tile[:, k * S + s:k * S + s + 1], axis=0
                                ),
                            )
                if "c" in stages:
                    # control: plain contiguous load of the same byte count
                    cap = bass.AP(inp, 0, [[S * L, P], [1, S * L]])
                    nc.sync.dma_start(out=t[:], in_=cap)
                if "m" in stages:
                    # keep element (p, s, l) iff l < L - p (the row's data len)
                    nc.gpsimd.affine_select(
                        out=t[:],
                        in_=t[:],
                        compare_op=mybir.AluOpType.is_gt,
                        fill=0.0,
                        base=L,
                        pattern=[[0, S], [-1, L]],
                        channel_multiplier=-1,
                    )
                if "s" in stages:
                    # band store: band row p -> flat 2049*(128k+p), per sample
                    oap = bass.AP(
                        out, (MAT + 1) * P * k, [[MAT + 1, P], [OUT_NP, S], [1, L]]
                    )
                    nc.sync.dma_start(out=oap, in_=t[:])
                if "z" in stages:
                    # zero parallelogram: matrix rows R=128k+1+j (j<cnt),
                    # cols [R-1-128k, R-1], length 128k+1, row starts affine
                    zl = P * k + 1
                    cnt = P if k < NB - 1 else P - 1
                    zap = bass.AP(
                        out,
                        (P * k + 1) * MAT,
                        [[MAT + 1, cnt], [OUT_NP, S], [1, zl]],
                    )
                    nc.scalar.dma_start(out=zap, in_=zt[:cnt, :S * zl])

            if rep_loop and repeat > 1:
                with tc.For_i(0, repeat, 1):
                    for k in range(NB):
                        gblock(k)
            else:
                for k in [k for _ in range(repeat) for k in range(NB)]:
                    gblock(k)
    nc.compile()
    return nc


def _build_fused(nc, bass, tile, mybir, inp, idxt, out, repeat, stages,
                 bufs, fold=False, rep_loop=False, msk2=None,
                 store_split=1, ilv=False):
    """Fused pipeline: per block, one folded indirect gather [P,S,L], one
    full-width affine_select into [P,S,2049] (kills the gather's junk tail
    AND materializes the zero triangle in-tile), one full-pitch band store
    (512 x 8196B descriptors). Band row r covers out flat
    [2049r, 2049(r+1)) = row r's data + row r+1's zero prefix; the union
    over r tiles the padded output exactly, so every byte is written once
    and no separate zero-fill DMAs are needed."""
    W = MAT + 1  # full-pitch band row width (2049)
    with tile.TileContext(nc) as tc:
        with (
            tc.tile_pool(name="band", bufs=bufs) as pool,
            tc.tile_pool(name="const", bufs=1) as cpool,
        ):
            idx_tile = cpool.tile([P, NB * S], mybir.dt.int32)
            nc.sync.dma_start(idx_tile[:], idxt[:, :])
            msk_tile = None
            if msk2 is not None:
                msk_tile = cpool.tile([P, WM2], mybir.dt.float32)
                nc.scalar.dma_start(msk_tile[:], msk2[:, :])
            # pre-zero band slots: the mask stage reads the full [P,S,W]
            # tile; cols >= L are stale on the first NB blocks
            for _ in range(bufs):
                t0 = pool.tile([P, S, W], mybir.dt.float32, tag="band")
                nc.vector.memset(t0[:], 0.0)
            sel = nc.gpsimd  # affine_select is gpsimd-only in this bass

            def block(k):
                L = MAT - P * k
                t = pool.tile([P, S, W], mybir.dt.float32, tag="band")
                if "g" in stages:
                    if fold:
                        nc.gpsimd.indirect_dma_start(
                            out=t[:, :, :L],
                            out_offset=None,
                            in_=inp[:],
                            in_offset=bass.IndirectOffsetOnAxis(
                                ap=idx_tile[:, k * S:(k + 1) * S], axis=0
                            ),
                        )
                    else:
                        for s in range(S):
                            nc.gpsimd.indirect_dma_start(
                                out=t[:, s, :L],
                                out_offset=None,
                                in_=inp[:],
                                in_offset=bass.IndirectOffsetOnAxis(
                                    ap=idx_tile[:, k * S + s:k * S + s + 1],
                                    axis=0,
                                ),
                            )
                elif "s" in stages or "m" in stages:
                    # ablation-only: mark tile written for the allocator
                    nc.vector.memset(t[:, :, :1], 0.0)
                if "m" in stages:
                    # keep (p, s, c) iff c < L - p (row's true data length);
                    # zero junk tail + zero suffix. Cols [0, L-P) are valid
                    # data for every p (junk starts at L-p >= L-P+1), so
                    # only the tail [c0, W) needs the mask pass.
                    if msk_tile is not None:
                        c0 = max(L - P, 0)
                        for s in range(S):
                            nc.vector.tensor_tensor(
                                out=t[:, s, c0:],
                                in0=t[:, s, c0:],
                                in1=msk_tile[:, P * k + c0:P * k + W],
                                op=mybir.AluOpType.mult,
                            )
                    else:
                        sel.affine_select(
                            out=t[:],
                            in_=t[:],
                            compare_op=mybir.AluOpType.is_gt,
                            fill=0.0,
                            base=L,
                            pattern=[[0, S], [-1, W]],
                            channel_multiplier=-1,
                        )
                if "s" in stages:
                    if ilv:
                        # interleaved layout: band row r sample s at flat
                        # (r*S + s)*W -> one contiguous 4*2049*4B = 32784B
                        # descriptor per partition, sequential across p
                        oap = bass.AP(
                            out, S * W * P * k, [[S * W, P], [1, S * W]]
                        )
                        eng = nc.sync if k % 2 == 0 else nc.scalar
                        eng.dma_start(out=oap, in_=t[:])
                    elif store_split > 1:
                        sc = S // store_split
                        for j in range(store_split):
                            oap = bass.AP(
                                out, W * P * k + OUT_NP * j * sc,
                                [[W, P], [OUT_NP, sc], [1, W]],
                            )
                            eng = nc.sync if j % 2 == 0 else nc.scalar
                            eng.dma_start(out=oap,
                                          in_=t[:, j * sc:(j + 1) * sc, :])
                    else:
                        oap = bass.AP(
                            out, W * P * k, [[W, P], [OUT_NP, S], [1, W]]
                        )
                        eng = nc.sync if k % 2 == 0 else nc.scalar
                        eng.dma_start(out=oap, in_=t[:])

            if rep_loop and repeat > 1:
                with tc.For_i(0, repeat, 1):
                    for k in range(NB):
                        block(k)
            else:
                for k in [k for _ in range(repeat) for k in range(NB)]:
                    block(k)
    nc.compile()
    return nc


def _build_grouped(nc, bass, tile, mybir, inp, mskt, out, off,
                   repeat, stages, bufs, leng="pool"):
    """Affine-only pipeline: per block, NG affine group loads (16 rows at
    constant stride L-16a, head-misaligned by h(b)=H-b(b-1)/2), one mask
    multiply per sample against a sliding master mask, then per-b-class
    band stores whose SBUF column offset h(b) absorbs the misalignment."""
    with tile.TileContext(nc) as tc:
        with (
            tc.tile_pool(name="band", bufs=bufs) as pool,
            tc.tile_pool(name="const", bufs=1) as cpool,
        ):
            msk_tile = cpool.tile([P, WM], mybir.dt.float32)
            nc.sync.dma_start(msk_tile[:], mskt[:, :])
            if "z" in stages:
                zt = cpool.tile([P, S * ZMAX], mybir.dt.float32)
                nc.vector.memset(zt[:], 0.0)
            # pre-zero the band slots so stale-bit NaNs can't leak through
            # the mask multiply (0 * NaN = NaN)
            ext = "x" in stages
            tw = WT if ext else MAT + H
            for _ in range(bufs):
                t0 = pool.tile([P, S, tw], mybir.dt.float32, tag="band")
                nc.vector.memset(t0[:], 0.0)
            for k in [k for _ in range(repeat) for k in range(NB)]:
                L = MAT - P * k
                W = WT if ext else L + H
                r0 = P * k
                t = pool.tile([P, S, W], mybir.dt.float32, tag="band")
                if "g" in stages:
                    for a in range(NG):
                        Lc = L + H - G * a
                        start = FPAD + int(off[r0 + G * a]) - H
                        iap = bass.AP(
                            inp, start,
                            [[L - G * a, G], [IN_NP, S], [1, Lc]],
                        )
                        le = (nc.gpsimd if leng == "pool"
                              else (nc.sync if a % 2 == 0 else nc.scalar))
                        le.dma_start(out=t[G * a:G * (a + 1), :, :Lc],
                                     in_=iap)
                if "m" in stages:
                    for s in range(S):
                        nc.vector.tensor_tensor(
                            out=t[:, s, :],
                            in0=t[:, s, :],
                            in1=msk_tile[:, P * k:P * k + W],
                            op=mybir.AluOpType.mult,
                        )
                if ext:
                    # full-pitch stores: band row r covers out flat
                    # [2049r, 2049(r+1)) = row r data + row r+1 zero prefix
                    # (incl. subdiagonal); union over r tiles the padded
                    # output exactly -> no separate zero fills
                    for b in range(G):
                        h = H - b * (b - 1) // 2
                        sb = t[b::G, :, h:h + MAT + 1]
                        oap = bass.AP(
                            out, (MAT + 1) * (r0 + b),
                            [[(MAT + 1) * G, NG], [OUT_NP, S], [1, MAT + 1]],
                        )
                        eng = nc.sync if b % 2 == 0 else nc.scalar
                        eng.dma_start(out=oap, in_=sb)
                elif "s" in stages:
                    for b in range(G):
                        h = H - b * (b - 1) // 2
                        sb = t[b::G, :, h:h + L]
                        oap = bass.AP(
                            out, (MAT + 1) * (r0 + b),
                            [[(MAT + 1) * G, NG], [OUT_NP, S], [1, L]],
                        )
                        eng = nc.sync if b % 2 == 0 else nc.scalar
                        eng.dma_start(out=oap, in_=sb)
                if "z" in stages:
                    zl = P * k + 1
                    cnt = P if k < NB - 1 else P - 1
                    zap = bass.AP(
                        out, (P * k + 1) * MAT,
                        [[MAT + 1, cnt], [OUT_NP, S], [1, zl]],
                    )
                    nc.scalar.dma_start(out=zap, in_=zt[:cnt, :S * zl])
    nc.compile()
    return nc


MODE = os.environ.get("TRIU_MODE", "fusedm")

# per-mode default build keyword tuning
DEFAULT_BUILD_KW = {
    "fusedm": {"bufs": 5, "store_split": 2},
    "fusedi": {"bufs": 5},
}

_NC = None


def _default_build(repeat: int = 1, rep_loop: bool = False):
    stages = {"grouped": "gmx", "fused": "gms", "fusedm": "gms",
              "fusedi": "gms"}.get(MODE, "gmsz")
    kw = DEFAULT_BUILD_KW.get(MODE, {})
    return _build_nc(repeat=repeat, mode=MODE, stages=stages,
                     rep_loop=rep_loop, **kw)


def _get_nc():
    global _NC
    if _NC is None:
        _NC = _default_build()
    return _NC


def _mask_array() -> np.ndarray:
    # master mask: m[p, x] = 1 iff x < MAT + H - p - D16(p % G)
    p = np.arange(P)[:, None]
    x = np.arange(WM)[None, :]
    b = p % G
    thr = MAT + H - p - b * (b - 1) // 2
    return (x < thr).astype(np.float32)


def _mask2_array() -> np.ndarray:
    # fused master mask: m[p, X] = 1 iff X < MAT - p; block k's slice is
    # [:, P*k : P*k + MAT+1] (tile col c valid iff c < MAT - P*k - p)
    p = np.arange(P)[:, None]
    x = np.arange(WM2)[None, :]
    return (x < MAT - p).astype(np.float32)


def make_in_maps(inputs: np.ndarray):
    """Shard + pad the [32, IN_N] input into 8 per-core in_maps."""
    assert inputs.shape == (BATCH, IN_N), inputs.shape
    x = np.ascontiguousarray(inputs, dtype=np.float32)
    xp = np.zeros((BATCH, IN_NP), dtype=np.float32)
    xp[:, FPAD:FPAD + IN_N] = x
    xp = xp.reshape(NCORES, S * IN_NP)

    off = _offsets()
    idx = np.zeros((P, NB * S), dtype=np.int32)
    for k in range(NB):
        for s in range(S):
            idx[:, k * S + s] = (
                FPAD + off[k * P:(k + 1) * P] + s * IN_NP
            ).astype(np.int32)
    msk = _mask_array()
    msk2 = _mask2_array()
    return [{"inp": xp[c][:, None], "idx": idx, "msk": msk, "msk2": msk2}
            for c in range(NCORES)]


def assemble_out(results, ilv: bool = False) -> np.ndarray:
    outs = []
    for c in range(NCORES):
        o = results[c]["out"]
        if ilv:
            # de-interleave: flat (r*S + s)*W + c -> per-sample band rows
            o = np.ascontiguousarray(
                o.reshape(MAT, S, MAT + 1).transpose(1, 0, 2)
            ).reshape(S, OUT_NP)[:, :OUT_N]
        else:
            o = o.reshape(S, OUT_NP)[:, :OUT_N]
        outs.append(o.reshape(S, MAT, MAT))
    return np.concatenate(outs, axis=0)


def kernel(inputs: np.ndarray) -> np.ndarray:
    from concourse.bass_utils import run_bass_kernel_spmd

    nc = _get_nc()
    in_maps = make_in_maps(np.asarray(inputs))
    res = run_bass_kernel_spmd(nc, in_maps, core_ids=list(range(NCORES)))
    return assemble_out(res.results, ilv=(MODE == "fusedi"))


if __name__ == "__main__":
    rng = np.random.default_rng(0)
    x = rng.standard_normal((BATCH, IN_N), dtype=np.float32)
    y = kernel(x)
    # numpy reference
    r, c = np.triu_indices(MAT)
    exp = np.zeros((BATCH, MAT, MAT), dtype=np.float32)
    exp[:, r, c] = x
    err = np.abs(y - exp).max()
    denom = max(np.abs(exp).max(), 1e-9)
    print("max abs err:", err, "rel:", err / denom)
    assert err == 0.0, "mismatch"
    print("OK")

